# revision 6
# baseline (speedup 1.0000x reference)
# Trainium2 Bass kernel for nn_ConceptGenerator (pointer-generator concept head).
#
# Strategy (8 NeuronCores, data-parallel over batch, 4 batch elems per core):
#   - cross-attention + FFN + transfer/diverter/generator heads computed per core
#     with bf16 matmuls (fp32 PSUM accumulation).
#   - The scatter_add into the 12500-wide extended vocab followed by a gather at
#     `target` is algebraically replaced by a gather:
#       probs[t,b,target] = gen_gate*softmax(gen_logits)[target]
#                         + sum_{s,g} copy_gate[g]*align[t,b,s]*(copy_seq[s,b,g]==target)
#     so the [T,B,12500] probs tensor is never materialized. The generator
#     softmax denominator is computed by streaming WgT tiles, and the numerator
#     logit by an indirect-DMA row gather of Wg at `target`.
#   - k-bias is dropped (softmax shift invariance); v-bias is applied after
#     attention (softmax weights sum to 1).
import sys

sys.path.insert(0, "/opt/trn_rl_repo")

import numpy as np
import ml_dtypes

P = 128
T, B, S, E, FF, C, V = 128, 32, 512, 512, 2048, 512, 12000
NCORES = 8
BL = B // NCORES          # batches per core
ROWS = BL * T             # 512 rows per core (row = b*T + t)
NT = 500                  # generator vocab tile width
NTILES = V // NT          # 24
EPS = 1e-5
SCALE = 1.0 / float(np.sqrt(np.float32(E)))

_CACHE = {}

bf16 = ml_dtypes.bfloat16


def _build_program():
    import concourse.bass as bass
    import concourse.tile as tile
    import concourse.mybir as mybir
    from concourse import bacc
    from concourse.masks import make_identity
    from contextlib import ExitStack

    dt = mybir.dt
    AFT = mybir.ActivationFunctionType
    ALU = mybir.AluOpType
    AX = mybir.AxisListType

    nc = bacc.Bacc(trn_type="TRN2")

    def din(name, shape, dtype):
        return nc.declare_dram_parameter(name, list(shape), dtype, isOutput=False)

    def dout(name, shape, dtype):
        return nc.declare_dram_parameter(name, list(shape), dtype, isOutput=True)

    # ---- inputs (per core) ----
    outs_f = din("outs_f", [BL, T, E], dt.float32)         # residual
    outsT = din("outsT", [BL, E, T], dt.bfloat16)          # matmul operand
    sntT = din("sntT", [BL, E, S], dt.bfloat16)
    maskadd = din("maskadd", [BL, S], dt.float32)          # -1e30 where padded
    c0f = din("c0f", [BL, S], dt.float32)
    c1f = din("c1f", [BL, S], dt.float32)
    tgtf = din("tgtf", [BL, T], dt.float32)
    tgti = din("tgti", [BL, T], dt.int32)
    wqT = din("wqT", [E, E], dt.bfloat16)
    wkT = din("wkT", [E, E], dt.bfloat16)
    wvT = din("wvT", [E, E], dt.bfloat16)
    woT = din("woT", [E, E], dt.bfloat16)
    w1T = din("w1T", [E, FF], dt.bfloat16)
    w2T = din("w2T", [FF, E], dt.bfloat16)
    wtT = din("wtT", [E, C], dt.bfloat16)
    wdT = din("wdT", [C, 3], dt.bfloat16)
    wgT = din("wgT", [C, V], dt.bfloat16)
    wgf = din("wgf", [V, C], dt.float32)                   # row gather source
    bgcol = din("bgcol", [V, 1], dt.float32)               # bias gather source
    bgbf = din("bgbf", [V], dt.bfloat16)                   # for exp(bg) bcast
    bqp = din("bqp", [P, E // P], dt.float32)
    bvp = din("bvp", [P, E // P], dt.float32)
    b1p = din("b1p", [P, FF // P], dt.float32)
    btp = din("btp", [P, C // P], dt.float32)
    bo_e = din("bo_e", [E], dt.float32)
    b2_e = din("b2_e", [E], dt.float32)
    l1g_e = din("l1g_e", [E], dt.float32)
    l1b_e = din("l1b_e", [E], dt.float32)
    l2g_e = din("l2g_e", [E], dt.float32)
    l2b_e = din("l2b_e", [E], dt.float32)
    bd3 = din("bd3", [3], dt.float32)

    # ---- outputs (per core) ----
    loss_o = dout("loss_o", [BL], dt.float32)
    outs2_o = dout("outs2_o", [T, BL, E], dt.float32)
    align_o = dout("align_o", [T, BL, S], dt.float32)

    def bcast_ap(src_ap):
        """DRAM AP [n] -> [P, n] with partition stride 0 (broadcast read)."""
        return bass.AP(tensor=src_ap.tensor, offset=src_ap.offset,
                       ap=[[0, P]] + [list(d) for d in src_ap.ap])

    with tile.TileContext(nc) as tc, ExitStack() as ctx:
        cw = ctx.enter_context(tc.tile_pool(name="cw", bufs=1))
        pb = ctx.enter_context(tc.tile_pool(name="pb", bufs=2))
        pb1 = ctx.enter_context(tc.tile_pool(name="pb1", bufs=1))
        px1 = ctx.enter_context(tc.tile_pool(name="px1", bufs=4))
        pbat = ctx.enter_context(tc.tile_pool(name="pbat", bufs=1))
        pht = ctx.enter_context(tc.tile_pool(name="pht", bufs=2))
        pgen = ctx.enter_context(tc.tile_pool(name="pgen", bufs=3))
        pmm = ctx.enter_context(tc.tile_pool(name="pmm", bufs=3, space="PSUM"))
        ptp = ctx.enter_context(tc.tile_pool(name="ptp", bufs=2, space="PSUM"))
        ptb = ctx.enter_context(tc.tile_pool(name="ptb", bufs=2, space="PSUM"))

        # ---------- constants ----------
        idf = cw.tile([P, P], dt.float32)
        make_identity(nc, idf)
        idb = cw.tile([P, P], dt.bfloat16)
        make_identity(nc, idb)
        ones_t = cw.tile([P, 1], dt.float32)
        nc.vector.memset(ones_t, 1.0)

        wq_t = cw.tile([P, 4, E], dt.bfloat16)
        nc.sync.dma_start(wq_t[:], wqT[:].rearrange("(ko p) f -> p ko f", p=P))
        wk_t = cw.tile([P, 4, E], dt.bfloat16)
        nc.sync.dma_start(wk_t[:], wkT[:].rearrange("(ko p) f -> p ko f", p=P))
        wv_t = cw.tile([P, 4, E], dt.bfloat16)
        nc.sync.dma_start(wv_t[:], wvT[:].rearrange("(ko p) f -> p ko f", p=P))
        wo_t = cw.tile([P, 4, E], dt.bfloat16)
        nc.sync.dma_start(wo_t[:], woT[:].rearrange("(ko p) f -> p ko f", p=P))
        w1_t = cw.tile([P, 4, FF], dt.bfloat16)
        nc.sync.dma_start(w1_t[:], w1T[:].rearrange("(ko p) f -> p ko f", p=P))
        w2_t = cw.tile([P, 16, E], dt.bfloat16)
        nc.sync.dma_start(w2_t[:], w2T[:].rearrange("(ko p) f -> p ko f", p=P))
        wt_t = cw.tile([P, 4, C], dt.bfloat16)
        nc.sync.dma_start(wt_t[:], wtT[:].rearrange("(ko p) f -> p ko f", p=P))
        wd_t = cw.tile([P, 4, 3], dt.bfloat16)
        nc.sync.dma_start(wd_t[:], wdT[:].rearrange("(ko p) f -> p ko f", p=P))

        bqs_t = cw.tile([P, 4], dt.float32)
        nc.sync.dma_start(bqs_t[:], bqp[:])
        nc.vector.tensor_scalar_mul(bqs_t[:], bqs_t[:], SCALE)  # (q+bq)*s
        bvp_t = cw.tile([P, 4], dt.float32)
        nc.sync.dma_start(bvp_t[:], bvp[:])
        b1p_t = cw.tile([P, 16], dt.float32)
        nc.sync.dma_start(b1p_t[:], b1p[:])
        btp_t = cw.tile([P, 4], dt.float32)
        nc.sync.dma_start(btp_t[:], btp[:])

        bo_t = cw.tile([P, E], dt.float32)
        nc.gpsimd.dma_start(out=bo_t[:], in_=bcast_ap(bo_e[:]))
        b2_t = cw.tile([P, E], dt.float32)
        nc.gpsimd.dma_start(out=b2_t[:], in_=bcast_ap(b2_e[:]))
        l1g_t = cw.tile([P, E], dt.float32)
        nc.gpsimd.dma_start(out=l1g_t[:], in_=bcast_ap(l1g_e[:]))
        l1b_t = cw.tile([P, E], dt.float32)
        nc.gpsimd.dma_start(out=l1b_t[:], in_=bcast_ap(l1b_e[:]))
        l2g_t = cw.tile([P, E], dt.float32)
        nc.gpsimd.dma_start(out=l2g_t[:], in_=bcast_ap(l2g_e[:]))
        l2b_t = cw.tile([P, E], dt.float32)
        nc.gpsimd.dma_start(out=l2b_t[:], in_=bcast_ap(l2b_e[:]))
        bd_t = cw.tile([P, 3], dt.float32)
        nc.gpsimd.dma_start(out=bd_t[:], in_=bcast_ap(bd3[:]))

        ebg_t = cw.tile([P, V], dt.bfloat16)
        nc.gpsimd.dma_start(out=ebg_t[:], in_=bcast_ap(bgbf[:]))
        nc.scalar.activation(ebg_t[:], ebg_t[:], AFT.Exp)

        tgtf_t = cw.tile([P, BL], dt.float32)
        nc.sync.dma_start(tgtf_t[:], tgtf[:].rearrange("b t -> t b"))
        tgti_t = cw.tile([P, BL], dt.int32)
        nc.sync.dma_start(tgti_t[:], tgti[:].rearrange("b t -> t b"))

        ccs = cw.tile([P, 2 * BL], dt.float32)      # copy-match partial sums
        loss_cols = cw.tile([P, BL], dt.float32)
        den_tiles = [cw.tile([P, NTILES], dt.float32, tag=f"den{m}", name=f"den{m}") for m in range(BL)]

        x1T_all = pbat.tile([P, 4, ROWS], dt.bfloat16, tag="x1T")
        o2T_all = pbat.tile([P, 4, ROWS], dt.bfloat16, tag="o2T")
        ocT_all = pbat.tile([P, 4, ROWS], dt.bfloat16, tag="ocT")

        x1_tiles = []

        # =============== attention + LN1, per batch element ===============
        for b in range(BL):
            sntT_b = pb.tile([P, 4, S], dt.bfloat16, tag="sntT")
            nc.sync.dma_start(sntT_b[:], sntT[b].rearrange("(ko p) s -> p ko s", p=P))
            outsT_b = pb.tile([P, 4, T], dt.bfloat16, tag="outsT")
            nc.sync.dma_start(outsT_b[:], outsT[b].rearrange("(ko p) t -> p ko t", p=P))
            ores_b = pb1.tile([P, E], dt.float32, tag="ores")
            nc.sync.dma_start(ores_b[:], outs_f[b])
            mask_b = pb1.tile([P, S], dt.float32, tag="maskb")
            nc.gpsimd.dma_start(out=mask_b[:], in_=bcast_ap(maskadd[b]))
            c0_b = pb1.tile([P, S], dt.float32, tag="c0b")
            nc.gpsimd.dma_start(out=c0_b[:], in_=bcast_ap(c0f[b]))
            c1_b = pb1.tile([P, S], dt.float32, tag="c1b")
            nc.gpsimd.dma_start(out=c1_b[:], in_=bcast_ap(c1f[b]))

            # qT[e,t] = sum_e' WqT[e',e] outsT[e',t]; scaled, +bq*s
            qT_b = pb.tile([P, 4, T], dt.bfloat16, tag="qT")
            for m in range(4):
                ps = ptp.tile([P, P], dt.float32, tag="tp32")
                for ko in range(4):
                    nc.tensor.matmul(ps[:], wq_t[:, ko, m * P:(m + 1) * P],
                                     outsT_b[:, ko, :], start=(ko == 0), stop=(ko == 3))
                nc.scalar.activation(qT_b[:, m, :], ps[:], AFT.Identity,
                                     bias=bqs_t[:, m:m + 1], scale=SCALE)
            # kT[e,s] (no bias: softmax shift-invariant)
            kT_b = pb.tile([P, 4, S], dt.bfloat16, tag="kT")
            for m in range(4):
                ps = pmm.tile([P, 512], dt.float32, tag="mm")
                for ko in range(4):
                    nc.tensor.matmul(ps[:], wk_t[:, ko, m * P:(m + 1) * P],
                                     sntT_b[:, ko, :], start=(ko == 0), stop=(ko == 3))
                nc.scalar.copy(kT_b[:, m, :], ps[:])
            # v[s,e] (bias applied later on attnT)
            v_b = pb.tile([P, 4, E], dt.bfloat16, tag="vS")
            for ms in range(4):
                ps = pmm.tile([P, 512], dt.float32, tag="mm")
                for ko in range(4):
                    nc.tensor.matmul(ps[:], sntT_b[:, ko, ms * P:(ms + 1) * P],
                                     wv_t[:, ko, :], start=(ko == 0), stop=(ko == 3))
                nc.scalar.copy(v_b[:, ms, :], ps[:])
            # scores[t,s] then masked softmax over s
            ps = pmm.tile([P, 512], dt.float32, tag="mm")
            for ko in range(4):
                nc.tensor.matmul(ps[:], qT_b[:, ko, :], kT_b[:, ko, :],
                                 start=(ko == 0), stop=(ko == 3))
            sc_b = pb1.tile([P, S], dt.float32, tag="sc")
            nc.scalar.copy(sc_b[:], ps[:])
            nc.vector.tensor_add(sc_b[:], sc_b[:], mask_b[:])
            st = pb.tile([P, 16], dt.float32, tag="st")
            nc.vector.reduce_max(st[:, 0:1], sc_b[:], axis=AX.X)
            nc.vector.tensor_scalar_mul(st[:, 1:2], st[:, 0:1], -1.0)
            w_b = pb.tile([P, S], dt.float32, tag="wf")
            nc.scalar.activation(w_b[:], sc_b[:], AFT.Exp,
                                 bias=st[:, 1:2], accum_out=st[:, 2:3])
            nc.vector.reciprocal(st[:, 3:4], st[:, 2:3])
            nc.vector.tensor_scalar_mul(w_b[:], w_b[:], st[:, 3:4])
            nc.sync.dma_start(align_o[:, b, :], w_b[:])
            # copy-prob partial sums vs target (uses w_b): (seq==tgt)*w summed over s
            junk = pb1.tile([P, S], dt.float32, tag="junk")
            nc.vector.scalar_tensor_tensor(
                out=junk[:], in0=c0_b[:], scalar=tgtf_t[:, b:b + 1], in1=w_b[:],
                op0=ALU.is_equal, op1=ALU.mult, accum_out=ccs[:, 2 * b:2 * b + 1])
            nc.vector.scalar_tensor_tensor(
                out=junk[:], in0=c1_b[:], scalar=tgtf_t[:, b:b + 1], in1=w_b[:],
                op0=ALU.is_equal, op1=ALU.mult, accum_out=ccs[:, 2 * b + 1:2 * b + 2])
            # wT[s,t]
            wT_b = pb.tile([P, 4, T], dt.bfloat16, tag="wT")
            for ks in range(4):
                pst = ptp.tile([P, P], dt.float32, tag="tp32")
                nc.tensor.transpose(pst[:], w_b[:, ks * P:(ks + 1) * P], idf[:])
                nc.scalar.copy(wT_b[:, ks, :], pst[:])
            # attnT[e,t] = sum_s v[s,e] wT[s,t]  (+bv per-partition)
            attnT_b = pb.tile([P, 4, T], dt.bfloat16, tag="attnT")
            for m in range(4):
                pst = ptp.tile([P, P], dt.float32, tag="tp32")
                for ks in range(4):
                    nc.tensor.matmul(pst[:], v_b[:, ks, m * P:(m + 1) * P],
                                     wT_b[:, ks, :], start=(ks == 0), stop=(ks == 3))
                nc.scalar.activation(attnT_b[:, m, :], pst[:], AFT.Identity,
                                     bias=bvp_t[:, m:m + 1])
            # x[t,f] = attn @ WoT + bo + outs; LN1
            ps = pmm.tile([P, 512], dt.float32, tag="mm")
            for ko in range(4):
                nc.tensor.matmul(ps[:], attnT_b[:, ko, :], wo_t[:, ko, :],
                                 start=(ko == 0), stop=(ko == 3))
            xf = pb.tile([P, E], dt.float32, tag="xf")
            nc.scalar.copy(xf[:], ps[:])
            nc.vector.tensor_add(xf[:], xf[:], bo_t[:])
            nc.vector.tensor_add(xf[:], xf[:], ores_b[:])
            nc.vector.reduce_sum(st[:, 4:5], xf[:], axis=AX.X)
            nc.vector.tensor_scalar_mul(st[:, 5:6], st[:, 4:5], 1.0 / E)
            nc.vector.tensor_scalar_sub(xf[:], xf[:], st[:, 5:6])
            nc.vector.scalar_tensor_tensor(
                out=junk[:], in0=xf[:], scalar=1.0, in1=xf[:],
                op0=ALU.mult, op1=ALU.mult, accum_out=st[:, 6:7])
            nc.vector.tensor_scalar(st[:, 7:8], st[:, 6:7], 1.0 / E, EPS,
                                    ALU.mult, ALU.add)
            nc.scalar.sqrt(st[:, 8:9], st[:, 7:8])
            nc.vector.reciprocal(st[:, 9:10], st[:, 8:9])
            x1_b = px1.tile([P, E], dt.float32, tag="x1")
            nc.vector.scalar_tensor_tensor(
                out=x1_b[:], in0=xf[:], scalar=st[:, 9:10], in1=l1g_t[:],
                op0=ALU.mult, op1=ALU.mult)
            nc.vector.tensor_add(x1_b[:], x1_b[:], l1b_t[:])
            x1_tiles.append(x1_b)
            for ks in range(4):
                pst = ptp.tile([P, P], dt.float32, tag="tp32")
                nc.tensor.transpose(pst[:], x1_b[:, ks * P:(ks + 1) * P], idf[:])
                nc.scalar.copy(x1T_all[:, ks, b * T:(b + 1) * T], pst[:])

        # =============== FFN + LN2 + transfer head (rows batched) ===============
        for m in range(BL):
            hT_m = pht.tile([P, 16, T], dt.bfloat16, tag="hTm")
            for f in range(16):
                pst = ptp.tile([P, P], dt.float32, tag="tp32")
                for ko in range(4):
                    nc.tensor.matmul(pst[:], w1_t[:, ko, f * P:(f + 1) * P],
                                     x1T_all[:, ko, m * T:(m + 1) * T],
                                     start=(ko == 0), stop=(ko == 3))
                nc.scalar.activation(hT_m[:, f, :], pst[:], AFT.Relu,
                                     bias=b1p_t[:, f:f + 1])
            ps = pmm.tile([P, 512], dt.float32, tag="mm")
            for ks in range(16):
                nc.tensor.matmul(ps[:], hT_m[:, ks, :], w2_t[:, ks, :],
                                 start=(ks == 0), stop=(ks == 15))
            yf = pb.tile([P, E], dt.float32, tag="yf")
            nc.scalar.copy(yf[:], ps[:])
            nc.vector.tensor_add(yf[:], yf[:], b2_t[:])
            nc.vector.tensor_add(yf[:], yf[:], x1_tiles[m][:])
            st2 = pb.tile([P, 16], dt.float32, tag="st2")
            nc.vector.reduce_sum(st2[:, 0:1], yf[:], axis=AX.X)
            nc.vector.tensor_scalar_mul(st2[:, 1:2], st2[:, 0:1], 1.0 / E)
            nc.vector.tensor_scalar_sub(yf[:], yf[:], st2[:, 1:2])
            junk2 = pb1.tile([P, E], dt.float32, tag="junk", name="junk2")
            nc.vector.scalar_tensor_tensor(
                out=junk2[:], in0=yf[:], scalar=1.0, in1=yf[:],
                op0=ALU.mult, op1=ALU.mult, accum_out=st2[:, 2:3])
            nc.vector.tensor_scalar(st2[:, 3:4], st2[:, 2:3], 1.0 / E, EPS,
                                    ALU.mult, ALU.add)
            nc.scalar.sqrt(st2[:, 4:5], st2[:, 3:4])
            nc.vector.reciprocal(st2[:, 5:6], st2[:, 4:5])
            o2f = pb.tile([P, E], dt.float32, tag="o2f")
            nc.vector.scalar_tensor_tensor(
                out=o2f[:], in0=yf[:], scalar=st2[:, 5:6], in1=l2g_t[:],
                op0=ALU.mult, op1=ALU.mult)
            nc.vector.tensor_add(o2f[:], o2f[:], l2b_t[:])
            nc.sync.dma_start(outs2_o[:, m, :], o2f[:])
            for ks in range(4):
                pst = ptp.tile([P, P], dt.float32, tag="tp32")
                nc.tensor.transpose(pst[:], o2f[:, ks * P:(ks + 1) * P], idf[:])
                nc.scalar.copy(o2T_all[:, ks, m * T:(m + 1) * T], pst[:])

        # ocT[c,rows] = tanh(WtT.T @ o2T + bt)
        for mc in range(4):
            ps = pmm.tile([P, 512], dt.float32, tag="mm")
            for ko in range(4):
                nc.tensor.matmul(ps[:], wt_t[:, ko, mc * P:(mc + 1) * P],
                                 o2T_all[:, ko, :], start=(ko == 0), stop=(ko == 3))
            nc.scalar.activation(ocT_all[:, mc, :], ps[:], AFT.Tanh,
                                 bias=btp_t[:, mc:mc + 1])

        # =============== generator softmax denominator (streamed) ===============
        for nt in range(NTILES):
            wg_nt = pgen.tile([P, 4, NT], dt.bfloat16, tag="wg")
            nc.sync.dma_start(
                wg_nt[:], wgT[:, nt * NT:(nt + 1) * NT].rearrange(
                    "(ko p) n -> p ko n", p=P))
            for m in range(BL):
                ps = pmm.tile([P, 512], dt.float32, tag="mm")
                for ko in range(4):
                    nc.tensor.matmul(ps[:, :NT], ocT_all[:, ko, m * T:(m + 1) * T],
                                     wg_nt[:, ko, :], start=(ko == 0), stop=(ko == 3))
                et = pgen.tile([P, NT], dt.float32, tag="expt")
                nc.scalar.activation(et[:], ps[:, :NT], AFT.Exp)
                nc.vector.scalar_tensor_tensor(
                    out=et[:], in0=et[:], scalar=1.0,
                    in1=ebg_t[:, nt * NT:(nt + 1) * NT],
                    op0=ALU.mult, op1=ALU.mult,
                    accum_out=den_tiles[m][:, nt:nt + 1])

        # =============== diverter gates + numerator + loss, per batch ===============
        for b in range(BL):
            # oc[t,c] via transpose of ocT
            oc_b = pb.tile([P, C], dt.bfloat16, tag="ocb")
            for ks in range(4):
                psb = ptb.tile([P, P], dt.bfloat16, tag="tpbf")
                nc.tensor.transpose(psb[:], ocT_all[:, ks, b * T:(b + 1) * T], idb[:])
                nc.scalar.copy(oc_b[:, ks * P:(ks + 1) * P], psb[:])
            # gates
            ps3 = ptp.tile([P, P], dt.float32, tag="tp32")
            for ko in range(4):
                nc.tensor.matmul(ps3[:, :3], ocT_all[:, ko, b * T:(b + 1) * T],
                                 wd_t[:, ko, :], start=(ko == 0), stop=(ko == 3))
            gf = pb.tile([P, 4], dt.float32, tag="gf")
            nc.scalar.copy(gf[:, :3], ps3[:, :3])
            nc.vector.tensor_add(gf[:, :3], gf[:, :3], bd_t[:])
            gs = pb.tile([P, 16], dt.float32, tag="gs")
            nc.vector.reduce_max(gs[:, 0:1], gf[:, :3], axis=AX.X)
            nc.vector.tensor_scalar_mul(gs[:, 1:2], gs[:, 0:1], -1.0)
            nc.scalar.activation(gf[:, :3], gf[:, :3], AFT.Exp,
                                 bias=gs[:, 1:2], accum_out=gs[:, 2:3])
            nc.vector.reciprocal(gs[:, 3:4], gs[:, 2:3])
            nc.vector.tensor_scalar_mul(gf[:, :3], gf[:, :3], gs[:, 3:4])
            # numerator logit: gather Wg rows + bg at target, dot with oc
            wgrow = pb1.tile([P, C], dt.float32, tag="wgrow")
            nc.gpsimd.indirect_dma_start(
                out=wgrow[:], out_offset=None, in_=wgf[:],
                in_offset=bass.IndirectOffsetOnAxis(ap=tgti_t[:, b:b + 1], axis=0))
            bgg = pb.tile([P, 1], dt.float32, tag="bgg")
            nc.gpsimd.indirect_dma_start(
                out=bgg[:], out_offset=None, in_=bgcol[:],
                in_offset=bass.IndirectOffsetOnAxis(ap=tgti_t[:, b:b + 1], axis=0))
            junk3 = pb1.tile([P, C], dt.float32, tag="junk", name="junk3")
            ls = pb.tile([P, 16], dt.float32, tag="ls")
            nc.vector.scalar_tensor_tensor(
                out=junk3[:], in0=oc_b[:], scalar=1.0, in1=wgrow[:],
                op0=ALU.mult, op1=ALU.mult, accum_out=ls[:, 0:1])
            nc.vector.tensor_add(ls[:, 1:2], ls[:, 0:1], bgg[:])
            nc.scalar.activation(ls[:, 2:3], ls[:, 1:2], AFT.Exp)
            nc.vector.reduce_sum(ls[:, 3:4], den_tiles[b][:], axis=AX.X)
            nc.vector.reciprocal(ls[:, 4:5], ls[:, 3:4])
            nc.vector.tensor_mul(ls[:, 5:6], ls[:, 2:3], gf[:, 0:1])
            nc.vector.tensor_mul(ls[:, 6:7], ls[:, 5:6], ls[:, 4:5])
            # copy part: cc0*copy_gate(g2) + cc1*map_gate(g1)
            nc.vector.tensor_mul(ls[:, 7:8], ccs[:, 2 * b:2 * b + 1], gf[:, 2:3])
            nc.vector.tensor_mul(ls[:, 8:9], ccs[:, 2 * b + 1:2 * b + 2], gf[:, 1:2])
            nc.vector.tensor_add(ls[:, 9:10], ls[:, 7:8], ls[:, 8:9])
            nc.vector.tensor_add(ls[:, 10:11], ls[:, 6:7], ls[:, 9:10])
            nc.vector.tensor_scalar_add(ls[:, 11:12], ls[:, 10:11], 1e-12)
            nc.scalar.activation(ls[:, 12:13], ls[:, 11:12], AFT.Ln)
            nc.vector.scalar_tensor_tensor(
                out=loss_cols[:, b:b + 1], in0=tgtf_t[:, b:b + 1], scalar=0.0,
                in1=ls[:, 12:13], op0=ALU.not_equal, op1=ALU.mult)

        # loss_b = -sum_t loss_cols[t,b]  (partition reduce via matmul with ones)
        ps4 = ptp.tile([P, P], dt.float32, tag="tp32")
        nc.tensor.matmul(ps4[:BL, :1], loss_cols[:], ones_t[:],
                         start=True, stop=True)
        lossf = cw.tile([P, 1], dt.float32)
        nc.scalar.activation(lossf[:BL, :], ps4[:BL, :1], AFT.Copy, scale=-1.0)
        nc.sync.dma_start(loss_o[:], lossf[:BL, 0])

    nc.compile()
    return nc


def _get_program():
    if "nc" not in _CACHE:
        _CACHE["nc"] = _build_program()
    return _CACHE["nc"]


def build_in_maps(**inputs):
    f32 = np.float32
    outs = np.asarray(inputs["outs"], f32)               # [T,B,E]
    snt = np.asarray(inputs["snt_state"], f32)           # [S,B,E]
    mask = np.asarray(inputs["snt_padding_mask"])        # [B,S] bool
    cs = np.asarray(inputs["copy_seq"])                  # [S,B,2] int
    tgt = np.asarray(inputs["target"])                   # [T,B] int
    Wq = np.asarray(inputs["Wq"], f32); bq = np.asarray(inputs["bq"], f32)
    Wk = np.asarray(inputs["Wk"], f32)
    Wv = np.asarray(inputs["Wv"], f32); bv = np.asarray(inputs["bv"], f32)
    Wo = np.asarray(inputs["Wo"], f32); bo = np.asarray(inputs["bo"], f32)
    W1 = np.asarray(inputs["W1"], f32); b1 = np.asarray(inputs["b1"], f32)
    W2 = np.asarray(inputs["W2"], f32); b2 = np.asarray(inputs["b2"], f32)
    Wt = np.asarray(inputs["Wt"], f32); bt = np.asarray(inputs["bt"], f32)
    Wg = np.asarray(inputs["Wg"], f32); bg = np.asarray(inputs["bg"], f32)
    Wd = np.asarray(inputs["Wd"], f32); bd = np.asarray(inputs["bd"], f32)
    l1g = np.asarray(inputs["ln1_g"], f32); l1b = np.asarray(inputs["ln1_b"], f32)
    l2g = np.asarray(inputs["ln2_g"], f32); l2b = np.asarray(inputs["ln2_b"], f32)

    def c(a):
        return np.ascontiguousarray(a)

    shared = {
        "wqT": c(Wq.T.astype(bf16)), "wkT": c(Wk.T.astype(bf16)),
        "wvT": c(Wv.T.astype(bf16)), "woT": c(Wo.T.astype(bf16)),
        "w1T": c(W1.T.astype(bf16)), "w2T": c(W2.T.astype(bf16)),
        "wtT": c(Wt.T.astype(bf16)), "wdT": c(Wd.T.astype(bf16)),
        "wgT": c(Wg.T.astype(bf16)), "wgf": c(Wg),
        "bgcol": c(bg[:, None]), "bgbf": c(bg.astype(bf16)),
        "bqp": c(bq.reshape(4, P).T), "bvp": c(bv.reshape(4, P).T),
        "b1p": c(b1.reshape(16, P).T), "btp": c(bt.reshape(4, P).T),
        "bo_e": bo, "b2_e": b2, "l1g_e": l1g, "l1b_e": l1b,
        "l2g_e": l2g, "l2b_e": l2b, "bd3": bd,
    }

    in_maps = []
    for core in range(NCORES):
        sl = slice(core * BL, (core + 1) * BL)
        m = dict(shared)
        m["outs_f"] = c(outs[:, sl, :].transpose(1, 0, 2))
        m["outsT"] = c(outs[:, sl, :].transpose(1, 2, 0).astype(bf16))
        m["sntT"] = c(snt[:, sl, :].transpose(1, 2, 0).astype(bf16))
        m["maskadd"] = c(np.where(mask[sl], np.float32(-1e30), np.float32(0.0)).astype(f32))
        m["c0f"] = c(cs[:, sl, 0].T.astype(f32))
        m["c1f"] = c(cs[:, sl, 1].T.astype(f32))
        m["tgtf"] = c(tgt[:, sl].T.astype(f32))
        m["tgti"] = c(tgt[:, sl].T.astype(np.int32))
        in_maps.append(m)
    return in_maps


_last_in_maps = None


def kernel(**inputs):
    from concourse.bass_utils import run_bass_kernel_spmd

    global _last_in_maps
    nc = _get_program()
    in_maps = build_in_maps(**inputs)
    _last_in_maps = in_maps

    res = run_bass_kernel_spmd(nc, in_maps, list(range(NCORES)))

    loss = np.concatenate([res.results[i]["loss_o"] for i in range(NCORES)], axis=0)
    outs2 = np.concatenate([res.results[i]["outs2_o"] for i in range(NCORES)], axis=1)
    align = np.concatenate([res.results[i]["align_o"] for i in range(NCORES)], axis=1)
    return (loss.astype(np.float32), outs2.astype(np.float32),
            align.astype(np.float32))


# revision 14
# speedup vs baseline: 35.4746x; 35.4746x over previous
# Trainium2 Bass kernel for nn_ConceptGenerator (pointer-generator concept head).
#
# Strategy (8 NeuronCores, data-parallel over batch, 4 batch elems per core):
#   - cross-attention + FFN + transfer/diverter/generator heads computed per core
#     with bf16 matmuls (fp32 PSUM accumulation).
#   - The scatter_add into the 12500-wide extended vocab followed by a gather at
#     `target` is algebraically replaced by a gather:
#       probs[t,b,target] = gen_gate*softmax(gen_logits)[target]
#                         + sum_{s,g} copy_gate[g]*align[t,b,s]*(copy_seq[s,b,g]==target)
#     so the [T,B,12500] probs tensor is never materialized. The generator
#     softmax denominator is computed by streaming WgT tiles, and the numerator
#     logit by an indirect-DMA row gather of Wg at `target` (issued up front so
#     the SWDGE latency is off the critical path).
#   - k-bias is dropped (softmax shift invariance); v-bias is applied after
#     attention (softmax weights sum to 1).
#   - PSUM evictions are split between DVE (tensor_scalar fused bias/relu) and
#     ACT (transcendentals) to balance engine load; matmuls are shaped to
#     maximize rhs free dim (fewer PE instructions).
import sys

sys.path.insert(0, "/opt/trn_rl_repo")

import numpy as np
import ml_dtypes

P = 128
T, B, S, E, FF, C, V = 128, 32, 512, 512, 2048, 512, 12000
NCORES = 8
BL = B // NCORES          # batches per core
ROWS = BL * T             # 512 rows per core (row = b*T + t)
NT = 500                  # generator vocab tile width
NTILES = V // NT          # 24
EPS = 1e-5
SCALE = 1.0 / float(np.sqrt(np.float32(E)))

_CACHE = {}

bf16 = ml_dtypes.bfloat16


def _build_program(reps=1):
    import concourse.bass as bass
    import concourse.tile as tile
    import concourse.mybir as mybir
    from concourse import bacc
    from concourse.masks import make_identity
    from contextlib import ExitStack

    dt = mybir.dt
    AFT = mybir.ActivationFunctionType
    ALU = mybir.AluOpType
    AX = mybir.AxisListType

    nc = bacc.Bacc(trn_type="TRN2")

    def din(name, shape, dtype):
        return nc.declare_dram_parameter(name, list(shape), dtype, isOutput=False)

    def dout(name, shape, dtype):
        return nc.declare_dram_parameter(name, list(shape), dtype, isOutput=True)

    # ---- inputs (per core) ----
    outs_f = din("outs_f", [BL, T, E], dt.float32)         # residual
    outsT = din("outsT", [E, ROWS], dt.bfloat16)           # [e, b*T+t]
    sntT = din("sntT", [BL, E, S], dt.bfloat16)
    maskadd = din("maskadd", [BL, S], dt.float32)          # -1e30 where padded
    c0f = din("c0f", [BL, S], dt.float32)
    c1f = din("c1f", [BL, S], dt.float32)
    tgtf = din("tgtf", [T, BL], dt.float32)
    tgti = din("tgti", [T, BL], dt.int32)
    wqT = din("wqT", [E, E], dt.bfloat16)
    wkT = din("wkT", [E, E], dt.bfloat16)
    wvT = din("wvT", [E, E], dt.bfloat16)
    woT = din("woT", [E, E], dt.bfloat16)
    w1T = din("w1T", [E, FF], dt.bfloat16)
    w2T = din("w2T", [FF, E], dt.bfloat16)
    wtT = din("wtT", [E, C], dt.bfloat16)
    wdT = din("wdT", [C, 3], dt.bfloat16)
    wgT = din("wgT", [C, V], dt.bfloat16)
    wgf = din("wgf", [V, C], dt.float32)                   # row gather source
    bgcol = din("bgcol", [V, 1], dt.float32)               # bias gather source
    bgbf = din("bgbf", [V], dt.bfloat16)                   # for exp(bg) bcast
    bqp = din("bqp", [P, E // P], dt.float32)
    bvp = din("bvp", [P, E // P], dt.float32)
    b1p = din("b1p", [P, FF // P], dt.float32)
    btp = din("btp", [P, C // P], dt.float32)
    bo_e = din("bo_e", [E], dt.float32)
    b2_e = din("b2_e", [E], dt.float32)
    l1g_e = din("l1g_e", [E], dt.float32)
    l1b_e = din("l1b_e", [E], dt.float32)
    l2g_e = din("l2g_e", [E], dt.float32)
    l2b_e = din("l2b_e", [E], dt.float32)
    bd3 = din("bd3", [3], dt.float32)

    # ---- outputs (per core) ----
    loss_o = dout("loss_o", [BL], dt.float32)
    outs2_o = dout("outs2_o", [T, BL, E], dt.float32)
    align_o = dout("align_o", [T, BL, S], dt.float32)

    def bcast_ap(src_ap):
        """DRAM AP [n] -> [P, n] with partition stride 0 (broadcast read)."""
        return bass.AP(tensor=src_ap.tensor, offset=src_ap.offset,
                       ap=[[0, P]] + [list(d) for d in src_ap.ap])

    with tile.TileContext(nc) as tc, ExitStack() as ctx:
      cw = ctx.enter_context(tc.tile_pool(name="cw", bufs=1))
      pb = ctx.enter_context(tc.tile_pool(name="pb", bufs=2))
      pb1 = ctx.enter_context(tc.tile_pool(name="pb1", bufs=1))
      px1 = ctx.enter_context(tc.tile_pool(name="px1", bufs=4))
      pg4 = ctx.enter_context(tc.tile_pool(name="pg4", bufs=4))
      pbat = ctx.enter_context(tc.tile_pool(name="pbat", bufs=1))
      pgen = ctx.enter_context(tc.tile_pool(name="pgen", bufs=2))
      pmm = ctx.enter_context(tc.tile_pool(name="pmm", bufs=3, space="PSUM"))
      ptp = ctx.enter_context(tc.tile_pool(name="ptp", bufs=2, space="PSUM"))
      ptb = ctx.enter_context(tc.tile_pool(name="ptb", bufs=2, space="PSUM"))
      for _rep in range(reps):
        # ---------- constants ----------
        idf = cw.tile([P, P], dt.float32)
        make_identity(nc, idf)
        idb = cw.tile([P, P], dt.bfloat16)
        make_identity(nc, idb)
        ones_t = cw.tile([P, 1], dt.float32)
        nc.vector.memset(ones_t, 1.0)

        wq_t = cw.tile([P, 4, E], dt.bfloat16)
        nc.sync.dma_start(wq_t[:], wqT[:].rearrange("(ko p) f -> p ko f", p=P))
        wk_t = cw.tile([P, 4, E], dt.bfloat16)
        nc.sync.dma_start(wk_t[:], wkT[:].rearrange("(ko p) f -> p ko f", p=P))
        wv_t = cw.tile([P, 4, E], dt.bfloat16)
        nc.sync.dma_start(wv_t[:], wvT[:].rearrange("(ko p) f -> p ko f", p=P))
        wo_t = cw.tile([P, 4, E], dt.bfloat16)
        nc.sync.dma_start(wo_t[:], woT[:].rearrange("(ko p) f -> p ko f", p=P))
        outsT_t = cw.tile([P, 4, ROWS], dt.bfloat16)
        nc.sync.dma_start(outsT_t[:], outsT[:].rearrange("(ko p) r -> p ko r", p=P))

        bqs_t = cw.tile([P, 4], dt.float32)
        nc.sync.dma_start(bqs_t[:], bqp[:])
        nc.vector.tensor_scalar_mul(bqs_t[:], bqs_t[:], SCALE)  # (q+bq)*s
        bvp_t = cw.tile([P, 4], dt.float32)
        nc.sync.dma_start(bvp_t[:], bvp[:])

        bo_t = cw.tile([P, E], dt.float32)
        nc.sync.dma_start(out=bo_t[:], in_=bcast_ap(bo_e[:]))
        l1g_t = cw.tile([P, E], dt.float32)
        nc.sync.dma_start(out=l1g_t[:], in_=bcast_ap(l1g_e[:]))
        l1b_t = cw.tile([P, E], dt.float32)
        nc.sync.dma_start(out=l1b_t[:], in_=bcast_ap(l1b_e[:]))

        tgtf_t = cw.tile([P, BL], dt.float32)
        nc.sync.dma_start(tgtf_t[:], tgtf[:])
        tgti_t = cw.tile([P, BL], dt.int32)
        nc.sync.dma_start(tgti_t[:], tgti[:])

        ccs = cw.tile([P, 2 * BL], dt.float32)      # copy-match partial sums
        loss_cols = cw.tile([P, BL], dt.float32)
        den_tiles = [cw.tile([P, NTILES], dt.float32, tag=f"den{m}",
                             name=f"den{m}") for m in range(BL)]

        # numerator gathers, issued up front (SWDGE latency off critical path)
        wgrow_tiles = []
        bgg_tiles = []
        for b in range(BL):
            wgrow = pg4.tile([P, C], dt.float32, tag="wgrow", name=f"wgrow{b}")
            nc.gpsimd.indirect_dma_start(
                out=wgrow[:], out_offset=None, in_=wgf[:],
                in_offset=bass.IndirectOffsetOnAxis(ap=tgti_t[:, b:b + 1], axis=0))
            wgrow_tiles.append(wgrow)
            bgg = pg4.tile([P, 1], dt.float32, tag="bgg", name=f"bgg{b}")
            nc.gpsimd.indirect_dma_start(
                out=bgg[:], out_offset=None, in_=bgcol[:],
                in_offset=bass.IndirectOffsetOnAxis(ap=tgti_t[:, b:b + 1], axis=0))
            bgg_tiles.append(bgg)

        x1T_all = pbat.tile([P, 4, ROWS], dt.bfloat16, tag="x1T")
        o2T_all = pbat.tile([P, 4, ROWS], dt.bfloat16, tag="o2T")
        ocT_all = pbat.tile([P, 4, ROWS], dt.bfloat16, tag="ocT")
        qT_all = pbat.tile([P, 4, ROWS], dt.bfloat16, tag="qT")

        # qT[e, row] for all rows at once (rhs free = 512)
        for m in range(4):
            ps = pmm.tile([P, 512], dt.float32, tag="mm")
            for ko in range(4):
                nc.tensor.matmul(ps[:], wq_t[:, ko, m * P:(m + 1) * P],
                                 outsT_t[:, ko, :], start=(ko == 0), stop=(ko == 3))
            nc.vector.tensor_scalar(qT_all[:, m, :], ps[:], SCALE,
                                    bqs_t[:, m:m + 1], ALU.mult, ALU.add)

        w1_t = cw.tile([P, 4, FF], dt.bfloat16)
        nc.gpsimd.dma_start(w1_t[:], w1T[:].rearrange("(ko p) f -> p ko f", p=P))
        w2_t = cw.tile([P, 16, E], dt.bfloat16)
        nc.gpsimd.dma_start(w2_t[:], w2T[:].rearrange("(ko p) f -> p ko f", p=P))
        wt_t = cw.tile([P, 4, C], dt.bfloat16)
        nc.gpsimd.dma_start(wt_t[:], wtT[:].rearrange("(ko p) f -> p ko f", p=P))
        wd_t = cw.tile([P, 4, 3], dt.bfloat16)
        nc.gpsimd.dma_start(wd_t[:], wdT[:].rearrange("(ko p) f -> p ko f", p=P))
        b1p_t = cw.tile([P, 16], dt.float32)
        nc.gpsimd.dma_start(b1p_t[:], b1p[:])
        btp_t = cw.tile([P, 4], dt.float32)
        nc.gpsimd.dma_start(btp_t[:], btp[:])
        b2_t = cw.tile([P, E], dt.float32)
        nc.gpsimd.dma_start(out=b2_t[:], in_=bcast_ap(b2_e[:]))
        l2g_t = cw.tile([P, E], dt.float32)
        nc.gpsimd.dma_start(out=l2g_t[:], in_=bcast_ap(l2g_e[:]))
        l2b_t = cw.tile([P, E], dt.float32)
        nc.gpsimd.dma_start(out=l2b_t[:], in_=bcast_ap(l2b_e[:]))
        bd_t = cw.tile([P, 3], dt.float32)
        nc.gpsimd.dma_start(out=bd_t[:], in_=bcast_ap(bd3[:]))

        ebg_t = cw.tile([P, V], dt.bfloat16)
        nc.gpsimd.dma_start(out=ebg_t[:], in_=bcast_ap(bgbf[:]))
        nc.scalar.activation(ebg_t[:], ebg_t[:], AFT.Exp)

        x1_tiles = []

        # =============== attention + LN1, per batch element ===============
        for b in range(BL):
            sntT_b = pb.tile([P, 4, S], dt.bfloat16, tag="sntT")
            nc.sync.dma_start(sntT_b[:], sntT[b].rearrange("(ko p) s -> p ko s", p=P))
            ores_b = pb1.tile([P, E], dt.float32, tag="ores")
            nc.sync.dma_start(ores_b[:], outs_f[b])
            mask_b = pb1.tile([P, S], dt.float32, tag="maskb")
            nc.sync.dma_start(out=mask_b[:], in_=bcast_ap(maskadd[b]))
            c0_b = pb1.tile([P, S], dt.float32, tag="c0b")
            nc.sync.dma_start(out=c0_b[:], in_=bcast_ap(c0f[b]))
            c1_b = pb1.tile([P, S], dt.float32, tag="c1b")
            nc.sync.dma_start(out=c1_b[:], in_=bcast_ap(c1f[b]))

            # kT[e,s] (no bias: softmax shift-invariant)
            kT_b = pb.tile([P, 4, S], dt.bfloat16, tag="kT")
            for m in range(4):
                ps = pmm.tile([P, 512], dt.float32, tag="mm")
                for ko in range(4):
                    nc.tensor.matmul(ps[:], wk_t[:, ko, m * P:(m + 1) * P],
                                     sntT_b[:, ko, :], start=(ko == 0), stop=(ko == 3))
                nc.scalar.copy(kT_b[:, m, :], ps[:])
            # v[s,e] (bias applied later on attnT)
            v_b = pb.tile([P, 4, E], dt.bfloat16, tag="vS")
            for ms in range(4):
                ps = pmm.tile([P, 512], dt.float32, tag="mm")
                for ko in range(4):
                    nc.tensor.matmul(ps[:], sntT_b[:, ko, ms * P:(ms + 1) * P],
                                     wv_t[:, ko, :], start=(ko == 0), stop=(ko == 3))
                nc.scalar.copy(v_b[:, ms, :], ps[:])
            # scores[t,s] then masked softmax over s
            ps = pmm.tile([P, 512], dt.float32, tag="mm")
            for ko in range(4):
                nc.tensor.matmul(ps[:], qT_all[:, ko, b * T:(b + 1) * T],
                                 kT_b[:, ko, :], start=(ko == 0), stop=(ko == 3))
            sc_b = pb1.tile([P, S], dt.float32, tag="sc")
            nc.vector.tensor_add(sc_b[:], ps[:], mask_b[:])  # evict + mask
            st = pb.tile([P, 16], dt.float32, tag="st")
            nc.vector.reduce_max(st[:, 0:1], sc_b[:], axis=AX.X)
            nc.vector.tensor_scalar_mul(st[:, 1:2], st[:, 0:1], -1.0)
            w_b = pb.tile([P, S], dt.float32, tag="wf")
            nc.scalar.activation(w_b[:], sc_b[:], AFT.Exp,
                                 bias=st[:, 1:2], accum_out=st[:, 2:3])
            nc.vector.reciprocal(st[:, 3:4], st[:, 2:3])
            nc.vector.tensor_scalar_mul(w_b[:], w_b[:], st[:, 3:4])
            nc.sync.dma_start(align_o[:, b, :], w_b[:])
            # copy-prob partial sums vs target (uses w_b)
            junk = pb1.tile([P, S], dt.float32, tag="junk")
            nc.vector.scalar_tensor_tensor(
                out=junk[:], in0=c0_b[:], scalar=tgtf_t[:, b:b + 1], in1=w_b[:],
                op0=ALU.is_equal, op1=ALU.mult, accum_out=ccs[:, 2 * b:2 * b + 1])
            nc.vector.scalar_tensor_tensor(
                out=junk[:], in0=c1_b[:], scalar=tgtf_t[:, b:b + 1], in1=w_b[:],
                op0=ALU.is_equal, op1=ALU.mult, accum_out=ccs[:, 2 * b + 1:2 * b + 2])
            # wT[s,t]
            wT_b = pb.tile([P, 4, T], dt.bfloat16, tag="wT")
            for ks in range(4):
                pst = ptp.tile([P, P], dt.float32, tag="tp32")
                nc.tensor.transpose(pst[:], w_b[:, ks * P:(ks + 1) * P], idf[:])
                nc.vector.tensor_copy(wT_b[:, ks, :], pst[:])
            # attnT[e,t] = sum_s v[s,e] wT[s,t]  (+bv per-partition)
            attnT_b = pb.tile([P, 4, T], dt.bfloat16, tag="attnT")
            for m in range(4):
                pst = ptp.tile([P, P], dt.float32, tag="tp32")
                for ks in range(4):
                    nc.tensor.matmul(pst[:], v_b[:, ks, m * P:(m + 1) * P],
                                     wT_b[:, ks, :], start=(ks == 0), stop=(ks == 3))
                nc.vector.tensor_scalar_add(attnT_b[:, m, :], pst[:],
                                            bvp_t[:, m:m + 1])
            # x[t,f] = attn @ WoT + bo + outs; LN1
            ps = pmm.tile([P, 512], dt.float32, tag="mm")
            for ko in range(4):
                nc.tensor.matmul(ps[:], attnT_b[:, ko, :], wo_t[:, ko, :],
                                 start=(ko == 0), stop=(ko == 3))
            xf = pb.tile([P, E], dt.float32, tag="xf")
            nc.vector.tensor_add(xf[:], ps[:], bo_t[:])    # evict + bias
            nc.vector.tensor_add(xf[:], xf[:], ores_b[:])
            nc.vector.reduce_sum(st[:, 4:5], xf[:], axis=AX.X)
            nc.vector.tensor_scalar_mul(st[:, 5:6], st[:, 4:5], 1.0 / E)
            nc.vector.tensor_scalar_sub(xf[:], xf[:], st[:, 5:6])
            nc.vector.scalar_tensor_tensor(
                out=junk[:], in0=xf[:], scalar=1.0, in1=xf[:],
                op0=ALU.mult, op1=ALU.mult, accum_out=st[:, 6:7])
            nc.vector.tensor_scalar(st[:, 7:8], st[:, 6:7], 1.0 / E, EPS,
                                    ALU.mult, ALU.add)
            nc.scalar.sqrt(st[:, 8:9], st[:, 7:8])
            nc.vector.reciprocal(st[:, 9:10], st[:, 8:9])
            x1_b = px1.tile([P, E], dt.float32, tag="x1")
            nc.vector.scalar_tensor_tensor(
                out=x1_b[:], in0=xf[:], scalar=st[:, 9:10], in1=l1g_t[:],
                op0=ALU.mult, op1=ALU.mult)
            nc.vector.tensor_add(x1_b[:], x1_b[:], l1b_t[:])
            x1_tiles.append(x1_b)
            for ks in range(4):
                pst = ptp.tile([P, P], dt.float32, tag="tp32")
                nc.tensor.transpose(pst[:], x1_b[:, ks * P:(ks + 1) * P], idf[:])
                nc.vector.tensor_copy(x1T_all[:, ks, b * T:(b + 1) * T], pst[:])

        # =============== FFN + LN2 + transfer head (rows batched) ===============
        hT_all = pbat.tile([P, 16, ROWS], dt.bfloat16, tag="hT")
        for f in range(16):
            ps = pmm.tile([P, 512], dt.float32, tag="mm")
            for ko in range(4):
                nc.tensor.matmul(ps[:], w1_t[:, ko, f * P:(f + 1) * P],
                                 x1T_all[:, ko, :], start=(ko == 0), stop=(ko == 3))
            nc.scalar.activation(hT_all[:, f, :], ps[:], AFT.Relu,
                                 bias=b1p_t[:, f:f + 1])
        for m in range(BL):
            ps = pmm.tile([P, 512], dt.float32, tag="mm")
            for ks in range(16):
                nc.tensor.matmul(ps[:], hT_all[:, ks, m * T:(m + 1) * T],
                                 w2_t[:, ks, :], start=(ks == 0), stop=(ks == 15))
            yf = pb1.tile([P, E], dt.float32, tag="yf")
            nc.vector.tensor_add(yf[:], ps[:], b2_t[:])
            nc.vector.tensor_add(yf[:], yf[:], x1_tiles[m][:])
            st2 = pb.tile([P, 16], dt.float32, tag="st2")
            nc.vector.reduce_sum(st2[:, 0:1], yf[:], axis=AX.X)
            nc.vector.tensor_scalar_mul(st2[:, 1:2], st2[:, 0:1], 1.0 / E)
            nc.vector.tensor_scalar_sub(yf[:], yf[:], st2[:, 1:2])
            junk2 = pb1.tile([P, E], dt.float32, tag="junk", name="junk2")
            nc.vector.scalar_tensor_tensor(
                out=junk2[:], in0=yf[:], scalar=1.0, in1=yf[:],
                op0=ALU.mult, op1=ALU.mult, accum_out=st2[:, 2:3])
            nc.vector.tensor_scalar(st2[:, 3:4], st2[:, 2:3], 1.0 / E, EPS,
                                    ALU.mult, ALU.add)
            nc.scalar.sqrt(st2[:, 4:5], st2[:, 3:4])
            nc.vector.reciprocal(st2[:, 5:6], st2[:, 4:5])
            o2f = pb.tile([P, E], dt.float32, tag="o2f")
            nc.vector.scalar_tensor_tensor(
                out=o2f[:], in0=yf[:], scalar=st2[:, 5:6], in1=l2g_t[:],
                op0=ALU.mult, op1=ALU.mult)
            nc.vector.tensor_add(o2f[:], o2f[:], l2b_t[:])
            nc.sync.dma_start(outs2_o[:, m, :], o2f[:])
            for ks in range(4):
                pst = ptp.tile([P, P], dt.float32, tag="tp32")
                nc.tensor.transpose(pst[:], o2f[:, ks * P:(ks + 1) * P], idf[:])
                nc.vector.tensor_copy(o2T_all[:, ks, m * T:(m + 1) * T], pst[:])

        # ocT[c,rows] = tanh(WtT.T @ o2T + bt)
        for mc in range(4):
            ps = pmm.tile([P, 512], dt.float32, tag="mm")
            for ko in range(4):
                nc.tensor.matmul(ps[:], wt_t[:, ko, mc * P:(mc + 1) * P],
                                 o2T_all[:, ko, :], start=(ko == 0), stop=(ko == 3))
            nc.scalar.activation(ocT_all[:, mc, :], ps[:], AFT.Tanh,
                                 bias=btp_t[:, mc:mc + 1])

        # ===== diverter gates + numerator (independent of the generator) =====
        gf_tiles = []
        ls_tiles = []
        for b in range(BL):
            # oc[t,c] via transpose of ocT
            oc_b = pb.tile([P, C], dt.bfloat16, tag="ocb")
            for ks in range(4):
                psb = ptb.tile([P, P], dt.bfloat16, tag="tpbf")
                nc.tensor.transpose(psb[:], ocT_all[:, ks, b * T:(b + 1) * T], idb[:])
                nc.vector.tensor_copy(oc_b[:, ks * P:(ks + 1) * P], psb[:])
            # gates
            ps3 = ptp.tile([P, P], dt.float32, tag="tp32")
            for ko in range(4):
                nc.tensor.matmul(ps3[:, :3], ocT_all[:, ko, b * T:(b + 1) * T],
                                 wd_t[:, ko, :], start=(ko == 0), stop=(ko == 3))
            gf = pg4.tile([P, 4], dt.float32, tag="gf", name=f"gf{b}")
            nc.vector.tensor_add(gf[:, :3], ps3[:, :3], bd_t[:])
            gs = pb.tile([P, 16], dt.float32, tag="gs")
            nc.vector.reduce_max(gs[:, 0:1], gf[:, :3], axis=AX.X)
            nc.vector.tensor_scalar_mul(gs[:, 1:2], gs[:, 0:1], -1.0)
            nc.scalar.activation(gf[:, :3], gf[:, :3], AFT.Exp,
                                 bias=gs[:, 1:2], accum_out=gs[:, 2:3])
            nc.vector.reciprocal(gs[:, 3:4], gs[:, 2:3])
            nc.vector.tensor_scalar_mul(gf[:, :3], gf[:, :3], gs[:, 3:4])
            gf_tiles.append(gf)
            # numerator logit: dot(oc, gathered Wg row) + gathered bg
            junk3 = pb1.tile([P, C], dt.float32, tag="junk", name="junk3")
            ls = pg4.tile([P, 16], dt.float32, tag="ls", name=f"ls{b}")
            nc.vector.scalar_tensor_tensor(
                out=junk3[:], in0=oc_b[:], scalar=1.0, in1=wgrow_tiles[b][:],
                op0=ALU.mult, op1=ALU.mult, accum_out=ls[:, 0:1])
            nc.vector.tensor_add(ls[:, 1:2], ls[:, 0:1], bgg_tiles[b][:])
            nc.scalar.activation(ls[:, 2:3], ls[:, 1:2], AFT.Exp)
            # copy part: cc0*copy_gate(g2) + cc1*map_gate(g1)
            nc.vector.tensor_mul(ls[:, 7:8], ccs[:, 2 * b:2 * b + 1], gf[:, 2:3])
            nc.vector.tensor_mul(ls[:, 8:9], ccs[:, 2 * b + 1:2 * b + 2], gf[:, 1:2])
            nc.vector.tensor_add(ls[:, 9:10], ls[:, 7:8], ls[:, 8:9])
            ls_tiles.append(ls)

        # =============== generator softmax denominator (streamed) ===============
        for nt in range(NTILES):
            wg_nt = pgen.tile([P, 4, NT], dt.bfloat16, tag="wg")
            nc.sync.dma_start(
                wg_nt[:], wgT[:, nt * NT:(nt + 1) * NT].rearrange(
                    "(ko p) n -> p ko n", p=P))
            for m in range(BL):
                ps = pmm.tile([P, 512], dt.float32, tag="mm")
                for ko in range(4):
                    nc.tensor.matmul(ps[:, :NT], ocT_all[:, ko, m * T:(m + 1) * T],
                                     wg_nt[:, ko, :], start=(ko == 0), stop=(ko == 3))
                et = pgen.tile([P, NT], dt.bfloat16, tag="expt")
                nc.scalar.activation(et[:], ps[:, :NT], AFT.Exp)
                nc.vector.scalar_tensor_tensor(
                    out=et[:], in0=et[:], scalar=1.0,
                    in1=ebg_t[:, nt * NT:(nt + 1) * NT],
                    op0=ALU.mult, op1=ALU.mult,
                    accum_out=den_tiles[m][:, nt:nt + 1])

        # ===== final loss assembly (needs generator denominators) =====
        for b in range(BL):
            gf = gf_tiles[b]
            ls = ls_tiles[b]
            nc.vector.reduce_sum(ls[:, 3:4], den_tiles[b][:], axis=AX.X)
            nc.vector.reciprocal(ls[:, 4:5], ls[:, 3:4])
            nc.vector.tensor_mul(ls[:, 5:6], ls[:, 2:3], gf[:, 0:1])
            nc.vector.tensor_mul(ls[:, 6:7], ls[:, 5:6], ls[:, 4:5])
            nc.vector.tensor_add(ls[:, 10:11], ls[:, 6:7], ls[:, 9:10])
            nc.vector.tensor_scalar_add(ls[:, 11:12], ls[:, 10:11], 1e-12)
            nc.scalar.activation(ls[:, 12:13], ls[:, 11:12], AFT.Ln)
            nc.vector.scalar_tensor_tensor(
                out=loss_cols[:, b:b + 1], in0=tgtf_t[:, b:b + 1], scalar=0.0,
                in1=ls[:, 12:13], op0=ALU.not_equal, op1=ALU.mult)

        # loss_b = -sum_t loss_cols[t,b]  (partition reduce via matmul with ones)
        ps4 = ptp.tile([P, P], dt.float32, tag="tp32")
        nc.tensor.matmul(ps4[:BL, :1], loss_cols[:], ones_t[:],
                         start=True, stop=True)
        lossf = cw.tile([P, 1], dt.float32)
        nc.scalar.activation(lossf[:BL, :], ps4[:BL, :1], AFT.Copy, scale=-1.0)
        nc.sync.dma_start(loss_o[:], lossf[:BL, 0])

    nc.compile()
    return nc


def _get_program(reps=1):
    key = f"nc{reps}"
    if key not in _CACHE:
        _CACHE[key] = _build_program(reps)
    return _CACHE[key]


def build_in_maps(**inputs):
    f32 = np.float32
    outs = np.asarray(inputs["outs"], f32)               # [T,B,E]
    snt = np.asarray(inputs["snt_state"], f32)           # [S,B,E]
    mask = np.asarray(inputs["snt_padding_mask"])        # [B,S] bool
    cs = np.asarray(inputs["copy_seq"])                  # [S,B,2] int
    tgt = np.asarray(inputs["target"])                   # [T,B] int
    Wq = np.asarray(inputs["Wq"], f32); bq = np.asarray(inputs["bq"], f32)
    Wk = np.asarray(inputs["Wk"], f32)
    Wv = np.asarray(inputs["Wv"], f32); bv = np.asarray(inputs["bv"], f32)
    Wo = np.asarray(inputs["Wo"], f32); bo = np.asarray(inputs["bo"], f32)
    W1 = np.asarray(inputs["W1"], f32); b1 = np.asarray(inputs["b1"], f32)
    W2 = np.asarray(inputs["W2"], f32); b2 = np.asarray(inputs["b2"], f32)
    Wt = np.asarray(inputs["Wt"], f32); bt = np.asarray(inputs["bt"], f32)
    Wg = np.asarray(inputs["Wg"], f32); bg = np.asarray(inputs["bg"], f32)
    Wd = np.asarray(inputs["Wd"], f32); bd = np.asarray(inputs["bd"], f32)
    l1g = np.asarray(inputs["ln1_g"], f32); l1b = np.asarray(inputs["ln1_b"], f32)
    l2g = np.asarray(inputs["ln2_g"], f32); l2b = np.asarray(inputs["ln2_b"], f32)

    def c(a):
        return np.ascontiguousarray(a)

    shared = {
        "wqT": c(Wq.T.astype(bf16)), "wkT": c(Wk.T.astype(bf16)),
        "wvT": c(Wv.T.astype(bf16)), "woT": c(Wo.T.astype(bf16)),
        "w1T": c(W1.T.astype(bf16)), "w2T": c(W2.T.astype(bf16)),
        "wtT": c(Wt.T.astype(bf16)), "wdT": c(Wd.T.astype(bf16)),
        "wgT": c(Wg.T.astype(bf16)), "wgf": c(Wg),
        "bgcol": c(bg[:, None]), "bgbf": c(bg.astype(bf16)),
        "bqp": c(bq.reshape(4, P).T), "bvp": c(bv.reshape(4, P).T),
        "b1p": c(b1.reshape(16, P).T), "btp": c(bt.reshape(4, P).T),
        "bo_e": bo, "b2_e": b2, "l1g_e": l1g, "l1b_e": l1b,
        "l2g_e": l2g, "l2b_e": l2b, "bd3": bd,
    }

    in_maps = []
    for core in range(NCORES):
        sl = slice(core * BL, (core + 1) * BL)
        m = dict(shared)
        m["outs_f"] = c(outs[:, sl, :].transpose(1, 0, 2))
        # [E, ROWS] with row = b*T + t
        m["outsT"] = c(outs[:, sl, :].transpose(2, 1, 0).reshape(E, ROWS).astype(bf16))
        m["sntT"] = c(snt[:, sl, :].transpose(1, 2, 0).astype(bf16))
        m["maskadd"] = c(np.where(mask[sl], np.float32(-1e30),
                                  np.float32(0.0)).astype(f32))
        m["c0f"] = c(cs[:, sl, 0].T.astype(f32))
        m["c1f"] = c(cs[:, sl, 1].T.astype(f32))
        m["tgtf"] = c(tgt[:, sl].astype(f32))
        m["tgti"] = c(tgt[:, sl].astype(np.int32))
        in_maps.append(m)
    return in_maps


_last_in_maps = None


def kernel(**inputs):
    from concourse.bass_utils import run_bass_kernel_spmd

    global _last_in_maps
    nc = _get_program()
    in_maps = build_in_maps(**inputs)
    _last_in_maps = in_maps

    res = run_bass_kernel_spmd(nc, in_maps, list(range(NCORES)))

    loss = np.concatenate([res.results[i]["loss_o"] for i in range(NCORES)], axis=0)
    outs2 = np.concatenate([res.results[i]["outs2_o"] for i in range(NCORES)], axis=1)
    align = np.concatenate([res.results[i]["align_o"] for i in range(NCORES)], axis=1)
    return (loss.astype(np.float32), outs2.astype(np.float32),
            align.astype(np.float32))
